# revision 18
# baseline (speedup 1.0000x reference)
"""NearbyAttention on 8 trn2 NeuronCores — v2 (bf16, interleaved).

Sharding: 2 heads per core (16 heads / 8 cores). Each core computes its
2 heads' QKV projections, block-sparse masked attention (42 of 100
[512q x 128k] blocks have any unmasked entry), and a partial output
projection fused across its 2 heads (K=128). Host sums the 8 partials
and adds the bias.

v2 changes vs v1:
- All operands bf16 (inputs/weights/masks/probs/partial outputs):
  halves DMA traffic, enables DVE 2x/4x modes. End-to-end numerics
  sim: max-rel 4.2e-3 (tolerance 2e-2).
- Score blocks processed in PAIRS sharing a 2-bank PSUM tile so one
  Act exp instruction covers [128, 1024] (42 activates instead of 84).
- Output projection contracts both heads in one matmul (K=128), with
  the per-head 1/rowsum folded into normT beforehand.
- Engine load balance: Act = exp only; DVE = masks, PSUM copies,
  normT; GpSimd = outproj PSUM->SBUF copies + broadcasts; stores on
  the scalar HWDGE ring, loads on the sync ring.
- Projections / attention / outproj emissions are interleaved at fine
  granularity so the PE never drains.

Numerics match the reference's where(mask, -fmax, dots) + softmax:
softmax without max-subtraction, masked entries killed by multiplying
exp(S) with a 0/1 mask tile. Query row 2560 is all-masked (reference
softmax gives uniform 1/n over all keys) and key column 2560 is masked
for every other query, so the device works on a clean 2560x2560
problem and the host computes output row 2560 directly.
"""

import numpy as np
import sys

sys.path.insert(0, "/opt/trn_rl_repo")

import ml_dtypes
import concourse.bass as bass
import concourse.bacc as bacc
import concourse.tile as tile
import concourse.mybir as mybir
from concourse import masks
from concourse.bass_utils import run_bass_kernel_spmd

N_CORES = 8
HEADS = 16
DH = 64
DIM = 1024
HPC = HEADS // N_CORES          # heads per core = 2
E = HPC * DH                    # per-core inner dim = 128
N_FULL = 2561
N = 2560                        # device seq len (row/col 2560 host-handled)
IC = 512                        # query chunk (free dim)
JT = 128                        # key tile (partition dim)
ND = DIM // 128                 # 8 contraction chunks for projections
N_ICS = N // IC                 # 5
N_JTS = N // JT                 # 20
SCALE = DH ** -0.5

F32 = mybir.dt.float32
BF = mybir.dt.bfloat16
BF_NP = ml_dtypes.bfloat16


def _block_schedule(mask):
    """From the bool mask (True = masked), compute the list of needed
    (ic, jt, uidx) blocks and the unique 0/1 mask tiles.
    uidx == -1 means the block is fully unmasked (skip the multiply)."""
    B = ~mask[:N, :N]  # True = attend
    uniq = {}
    tiles = []
    sched = []
    for ic in range(N_ICS):
        for jt in range(N_JTS):
            blk = B[ic * IC:(ic + 1) * IC, jt * JT:(jt + 1) * JT].T  # [128j, 512i]
            if not blk.any():
                continue
            if blk.all():
                sched.append((ic, jt, -1))
                continue
            key = blk.tobytes()
            if key not in uniq:
                uniq[key] = len(uniq)
                tiles.append(blk.astype(np.float32))
            sched.append((ic, jt, uniq[key]))
    mb = np.stack(tiles) if tiles else np.zeros((1, JT, IC), np.float32)
    return sched, mb


def _build(sched, n_mb):
    nc = bacc.Bacc("TRN2", target_bir_lowering=False, debug=False,
                   num_devices=N_CORES)

    # inputs pre-arranged on host: [128p, ND, N] with (p, d, n) = x.T[d*128+p, n]
    qT = nc.dram_tensor("qT", [128, ND, N], BF, kind="ExternalInput").ap()
    kT = nc.dram_tensor("kT", [128, ND, N], BF, kind="ExternalInput").ap()
    vT = nc.dram_tensor("vT", [128, ND, N], BF, kind="ExternalInput").ap()
    wq = nc.dram_tensor("wq", [128, ND, E], BF, kind="ExternalInput").ap()
    wk = nc.dram_tensor("wk", [128, ND, E], BF, kind="ExternalInput").ap()
    wv = nc.dram_tensor("wv", [128, ND, E], BF, kind="ExternalInput").ap()
    wo = nc.dram_tensor("wo", [E, DIM], BF, kind="ExternalInput").ap()
    mb = nc.dram_tensor("mb", [n_mb, JT, IC], BF, kind="ExternalInput").ap()
    out = nc.dram_tensor("out", [N, DIM], BF, kind="ExternalOutput").ap()

    by_ic = {}
    for ic, jt, u in sched:
        by_ic.setdefault(ic, []).append((jt, u))

    with tile.TileContext(nc) as tc:
        with (
            tc.tile_pool(name="consts", bufs=1) as consts,
            tc.tile_pool(name="load", bufs=6) as loadp,
            tc.tile_pool(name="big", bufs=1) as bigp,
            tc.tile_pool(name="vt", bufs=2) as vtp,
            tc.tile_pool(name="pt", bufs=3) as ptp,
            tc.tile_pool(name="sm", bufs=4) as smp,
            tc.tile_pool(name="ot", bufs=3) as otp,
            tc.tile_pool(name="mmps", bufs=2, space="PSUM") as mmps,
            tc.tile_pool(name="pops", bufs=1, space="PSUM") as pops,
            tc.tile_pool(name="sps", bufs=1, space="PSUM") as sps,
            tc.tile_pool(name="accps", bufs=2, space="PSUM") as accps,
        ):
            # ---- constants ----
            ident = consts.tile([128, 128], BF)
            masks.make_identity(nc, ident[:])


            w_sb = {}
            for name, ap in (("wq", wq), ("wk", wk), ("wv", wv)):
                t = consts.tile([128, ND, E], BF, tag=f"w_{name}")
                nc.gpsimd.dma_start(t[:], ap[:])
                w_sb[name] = t
            wo_sb = consts.tile([E, DIM], BF, name="wo_sb")
            nc.gpsimd.dma_start(wo_sb[:], wo[:])

            mb_sb = consts.tile([JT, n_mb, IC], BF)
            for u in range(n_mb):
                nc.gpsimd.dma_start(mb_sb[:, u, :], mb[u])

            # qhT/khT [e=128, n]: e on partitions (head0 rows 0:64, head1 64:128)
            qhT = bigp.tile([128, N], BF, tag="qhT")
            khT = bigp.tile([128, N], BF, tag="khT")
            # vh1 [j=128, jt, 130]: per j-tile [vh_h0 | 1 | vh_h1 | 1]
            vh1 = bigp.tile([JT, N_JTS, 130], BF, tag="vh1")
            nc.vector.memset(vh1[:, :, 64:65], 1.0)
            nc.vector.memset(vh1[:, :, 129:130], 1.0)
            # normT [e=128, n]: (attnT / rowsum), both heads stacked
            normT = bigp.tile([128, N], BF, tag="normT")

            in_sb = {}

            def emit_load(i):
                # two half-contraction DMAs per tensor: the first proj
                # matmuls (d<4) only depend on the first half landing.
                for name, src in (("q", qT), ("k", kT), ("v", vT)):
                    t = loadp.tile([128, ND, IC], BF, tag="ld",
                                   name=f"in_{name}{i}")
                    h2 = ND // 2
                    nc.sync.dma_start(t[:, 0:h2, :],
                                      src[:, 0:h2, i * IC:(i + 1) * IC])
                    nc.sync.dma_start(t[:, h2:ND, :],
                                      src[:, h2:ND, i * IC:(i + 1) * IC])
                    in_sb[(name, i)] = t

            # ---- emission quanta ----

            def proj_quanta(i):
                """Quanta (callables) for chunk i's projections."""
                quanta = []

                def mk_qk(name, w, dst):
                    def emit():
                        src = in_sb.pop((name[1], i))
                        ps = mmps.tile([128, IC], F32, tag="mm",
                                       name=f"ps_{name}{i}")
                        for d in range(ND):
                            nc.tensor.matmul(ps[:], w[:, d, :], src[:, d, :],
                                             start=(d == 0), stop=(d == ND - 1))
                        nc.vector.tensor_copy(dst[:, i * IC:(i + 1) * IC], ps[:])
                    return emit

                quanta.append(mk_qk("wq", w_sb["wq"], qhT))
                quanta.append(mk_qk("wk", w_sb["wk"], khT))

                vt_sb = vtp.tile([128, IC], BF, tag="vt", name=f"vt{i}")

                def emit_v():
                    src = in_sb.pop(("v", i))
                    ps = mmps.tile([128, IC], F32, tag="mm", name=f"ps_wv{i}")
                    for d in range(ND):
                        nc.tensor.matmul(ps[:], w_sb["wv"][:, d, :], src[:, d, :],
                                         start=(d == 0), stop=(d == ND - 1))
                    nc.vector.tensor_copy(vt_sb[:], ps[:])
                quanta.append(emit_v)

                def mk_tr(j4):
                    def emit():
                        jt = i * (IC // JT) + j4
                        tp = mmps.tile([128, JT], BF, tag="mm", name=f"tp{jt}")
                        nc.tensor.matmul(tp[:], vt_sb[:, j4 * JT:(j4 + 1) * JT],
                                         ident[:], is_transpose=True)
                        nc.vector.tensor_copy(vh1[:, jt, 0:DH], tp[:, 0:DH])
                        nc.vector.tensor_copy(vh1[:, jt, 65:65 + DH],
                                              tp[:, DH:2 * DH])
                    return emit

                for j4 in range(IC // JT):
                    quanta.append(mk_tr(j4))
                return quanta

            def outproj_quanta(c):
                quanta = []

                def mk(it):
                    def emit():
                        po = pops.tile([128, 2, 512], F32, tag="po",
                                       name=f"po{it}")
                        for oc in range(2):
                            nc.tensor.matmul(po[:, oc, :],
                                             normT[:, it * JT:(it + 1) * JT],
                                             wo_sb[:, oc * 512:(oc + 1) * 512],
                                             start=True, stop=True)
                        ot = otp.tile([128, 2, 512], BF, tag="ot",
                                      name=f"ot{it}")
                        # alternate the PSUM->SBUF copy between Act and DVE
                        if it % 2 == 0:
                            nc.scalar.copy(ot[:], po[:])
                        else:
                            nc.vector.tensor_copy(ot[:], po[:])
                        nc.scalar.dma_start(out[it * JT:(it + 1) * JT, :],
                                            ot[:])
                    return emit

                for j4 in range(IC // JT):
                    quanta.append(mk(c * (IC // JT) + j4))
                return quanta

            def attn_emitters(ic):
                """Per-pair emitters + a finalizer. Pair = up to 2 score
                blocks sharing one 2-bank PSUM tile and one exp."""
                jts = by_ic[ic]
                prs = [jts[t:t + 2] for t in range(0, len(jts), 2)]
                accs = [accps.tile([65, IC], F32, tag="acc",
                                   name=f"acc{ic}_{h}") for h in range(HPC)]
                nblk = len(jts)
                state = {"pend": [], "done": 0}

                def emit_pv(pts, blks):
                    base = state["done"]
                    for h in range(HPC):
                        for s, (jt, u) in enumerate(blks):
                            b = base + s
                            nc.tensor.matmul(
                                accs[h][:], vh1[:, jt, 65 * h:65 * h + 65],
                                pts[h][:, s, :],
                                start=(b == 0), stop=(b == nblk - 1))
                    state["done"] = base + len(blks)

                def mk_pair(t):
                    def emit():
                        blks = prs[t]
                        cur = []
                        # scores: h0/h1 use partition rows 0:64 / 64:128 ->
                        # automatic PE row-tiling; emit h0a,h1a,h0b,h1b
                        sp_h = [sps.tile([JT, 2, IC], F32, tag="s",
                                         name=f"s{ic}_{t}_{h}")
                                for h in range(HPC)]
                        for s, (jt, u) in enumerate(blks):
                            for h in range(HPC):
                                nc.tensor.matmul(
                                    sp_h[h][:, s, :],
                                    khT[h * DH:(h + 1) * DH,
                                        jt * JT:(jt + 1) * JT],
                                    qhT[h * DH:(h + 1) * DH,
                                        ic * IC:(ic + 1) * IC],
                                    start=True, stop=True)
                        for h in range(HPC):
                            pt = ptp.tile([JT, 2, IC], BF, tag="pt",
                                          name=f"pt{ic}_{t}_{h}")
                            ns = len(blks)
                            nc.scalar.activation(
                                pt[:, 0:ns, :], sp_h[h][:, 0:ns, :],
                                mybir.ActivationFunctionType.Exp, scale=SCALE)
                            for s, (jt, u) in enumerate(blks):
                                if u >= 0:
                                    nc.vector.tensor_mul(
                                        pt[:, s, :], pt[:, s, :],
                                        mb_sb[:, u, :])
                            cur.append(pt)
                        state["pend"].append((cur, blks))
                        if len(state["pend"]) > 1:
                            emit_pv(*state["pend"].pop(0))
                    return emit

                def finalize():
                    for p in state["pend"]:
                        emit_pv(*p)
                    state["pend"].clear()
                    # rowsum copy (Act) -> reciprocal (DVE) -> broadcast as
                    # a PE outer-product ones^T x rc into PSUM -> normT mul
                    # (DVE, reading both PSUM operands). bc2 lives in the
                    # pops ring, which also orders outproj(ic) after it.
                    rcs = []
                    for h in range(HPC):
                        rs = smp.tile([1, IC], F32, tag="rs",
                                      name=f"rs{ic}_{h}")
                        nc.scalar.copy(rs[:], accs[h][64:65, :])
                        rcs.append(rs)
                    bcs = []
                    for h in range(HPC):
                        rc = smp.tile([1, IC], F32, tag="rc",
                                      name=f"rc{ic}_{h}")
                        nc.vector.reciprocal_approx_fast(rc[:], rcs[h][:])
                        rcs[h] = rc
                    for h in range(HPC):
                        bc = smp.tile([DH, IC], F32, tag="bc",
                                      name=f"bc{ic}_{h}")
                        nc.gpsimd.partition_broadcast(bc[:], rcs[h][:])
                        bcs.append(bc)
                    for h in range(HPC):
                        nc.vector.tensor_mul(
                            normT[h * DH:(h + 1) * DH,
                                  ic * IC:(ic + 1) * IC],
                            accs[h][0:DH, :], bcs[h][:])

                return [mk_pair(t) for t in range(len(prs))], finalize

            # ---- interleaved schedule ----
            # steady state for chunk i: attn(i) pairs round-robined with
            # proj(i+1) and outproj(i-1) quanta; loads stay 2 chunks ahead.
            emit_load(0)
            emit_load(1)
            # warm up the PE clock while the first loads land
            for wi in range(12):
                wps = mmps.tile([128, JT], BF, tag="mm", name=f"warm{wi}")
                nc.tensor.matmul(wps[:], ident[:], ident[:],
                                 is_transpose=True)
            for q in proj_quanta(0):
                q()

            for i in range(N_ICS):
                pairs, finalize = attn_emitters(i)
                other = []
                if i + 2 < N_ICS:
                    other.append(lambda i=i: emit_load(i + 2))
                if i + 1 < N_ICS:
                    other.extend(proj_quanta(i + 1))
                # outproj(i-1) depends on finalize(i-1)'s normT, which
                # completes a few us into attn(i): emit those quanta late.
                if i > 0:
                    other.extend(outproj_quanta(i - 1))
                # distribute `other` quanta between pair emissions
                npr = len(pairs)
                k = 0
                for t, p in enumerate(pairs):
                    p()
                    want = ((t + 1) * len(other)) // npr
                    while k < want:
                        other[k]()
                        k += 1
                while k < len(other):
                    other[k]()
                    k += 1
                finalize()
            for q in outproj_quanta(N_ICS - 1):
                q()

    nc.compile()
    return nc


def _rearr(xT):
    # [DIM, N] -> [128, ND, N] bf16 with (p, d, n) = xT[d*128+p, n]
    return np.ascontiguousarray(
        xT.reshape(ND, 128, -1).transpose(1, 0, 2)).astype(BF_NP)


_CACHE = {}


def kernel(q, k, v, Wq, Wk, Wv, Wo, bo, mask_block, _trace=False):
    q = np.asarray(q); k = np.asarray(k); v = np.asarray(v)
    Wq = np.asarray(Wq, np.float32); Wk = np.asarray(Wk, np.float32)
    Wv = np.asarray(Wv, np.float32); Wo = np.asarray(Wo, np.float32)
    bo = np.asarray(bo, np.float32)
    mask = np.asarray(mask_block)
    b, n, d = q.shape
    assert (b, n, d) == (1, N_FULL, DIM)

    sched, mbt = _block_schedule(mask)
    n_mb = mbt.shape[0]

    key = (tuple(sched), n_mb)
    if key not in _CACHE:
        _CACHE[key] = _build(sched, n_mb)
    nc = _CACHE[key]

    qTb = _rearr(q[0, :N].T.astype(np.float32))
    kTb = _rearr(k[0, :N].T.astype(np.float32))
    vTb = _rearr(v[0, :N].T.astype(np.float32))
    mbb = mbt.astype(BF_NP)

    in_maps = []
    for c in range(N_CORES):
        sl = slice(c * E, (c + 1) * E)
        in_maps.append({
            "qT": qTb, "kT": kTb, "vT": vTb,
            "wq": _rearr(np.ascontiguousarray(Wq[:, sl])),
            "wk": _rearr(np.ascontiguousarray(Wk[:, sl])),
            "wv": _rearr(np.ascontiguousarray(Wv[:, sl])),
            "wo": np.ascontiguousarray(Wo[sl, :]).astype(BF_NP),
            "mb": mbb,
        })

    res = run_bass_kernel_spmd(
        nc, in_maps, core_ids=list(range(N_CORES)),
        trace=_trace, trace_cores=list(range(N_CORES)) if _trace else None)

    acc = res.results[0]["out"].astype(np.float32)
    for c in range(1, N_CORES):
        acc = acc + res.results[c]["out"].astype(np.float32)
    outf = np.empty((1, N_FULL, DIM), np.float32)
    outf[0, :N] = acc + bo

    # all-masked rows (row 2560): reference softmax is uniform over all keys
    am = np.where(mask.all(axis=1))[0]
    if am.size:
        vmean = v[0].astype(np.float32).mean(axis=0)
        row = (vmean @ Wv) @ Wo + bo
        outf[0, am] = row
    if _trace:
        kernel._last_exec_ns = res.exec_time_ns
        kernel._last_res = res
    return outf
